# revision 73
# baseline (speedup 1.0000x reference)
"""GQA causal attention (llama3-style RoPE) on 8 TRN2 NeuronCores.

Sharding: tensor-parallel over heads. Core c gets q-heads 4c..4c+3 and
kv-head c (GQA groups intact), plus the matching row-block of wo.T.
Each core computes a full [S, D] partial of the output projection;
the host sums the 8 partials (the "all-reduce" of the row-sharded wo).

Per-core pipeline (bf16/f16 everywhere; layouts picked so the only
transpose is a cheap PE transpose of y):
  qkvT[col, s]   = wqkvT.T @ xT             (K-tiled bf16 matmuls)
  RoPE on qT/kT  (two heads per pass via 128-row replicated cos/sin)
  sT[sk, sq]     = kT.T @ qT                (K=64 f16, causal-trimmed)
  eT             = exp(sT/8) * causal_mask  (ACT exp, DVE diag mask)
  y[sq, 65]      = eT.T @ v_aug             (flipped AV: out free size is
                                             only 65, half the PE cycles of
                                             the [65, sq] orientation, and
                                             col 64 = softmax denominator)
  y2             = y[:, 0:64] * recip(y[:, 64])   (per-partition scalar)
  yT             = PE-transpose(y2)
  out[sq, d]     = yT.T @ woT               (partial; host sums cores)
"""

import sys

for _p in ("/opt/trn_rl_repo", "/root/.axon_site/_ro/trn_rl_repo"):
    if _p not in sys.path:
        sys.path.insert(0, _p)

import numpy as np
import ml_dtypes

import concourse.bass as bass
import concourse.bacc as bacc
import concourse.mybir as mybir
import concourse.tile as tile

BF16 = ml_dtypes.bfloat16

S = 2048
D = 2048
HD = 64
NH = 32
NKV = 8
NCORES = 8
QH = NH // NCORES            # 4 local q heads
QCOLS = QH * HD              # 256
KVCOLS = 2 * HD              # 128 (k and v, one kv head)
P = 128                      # partitions
NK = D // P                  # 16 contraction tiles
NSQ = S // P                 # 16 seq tiles of 128
NCH = 4                      # seq chunks of 512
CH = 512

_CACHE = {}


def _build():
    mm_dt = mybir.dt.bfloat16
    f16 = mybir.dt.float16
    f32 = mybir.dt.float32

    nc = bacc.Bacc()
    xt_d = nc.dram_tensor("xt", [D, S], mm_dt, kind="ExternalInput")
    wqkvt_d = nc.dram_tensor("wqkvt", [D, QCOLS + KVCOLS], mm_dt, kind="ExternalInput")
    wot_d = nc.dram_tensor("wot", [QCOLS, D], mm_dt, kind="ExternalInput")
    cos_d = nc.dram_tensor("cos128", [P, S], f16, kind="ExternalInput")
    swap_d = nc.dram_tensor("swap128", [P, S], f16, kind="ExternalInput")
    masks_d = nc.dram_tensor("masks", [P, P], f16, kind="ExternalInput")
    ident_d = nc.dram_tensor("ident", [HD, HD], f16, kind="ExternalInput")
    ident128_d = nc.dram_tensor("ident128", [P, P], mm_dt, kind="ExternalInput")
    out_d = nc.dram_tensor("out", [S, D], mm_dt, kind="ExternalOutput")

    with tile.TileContext(nc) as tc:
        with (
            tc.tile_pool(name="const", bufs=1) as cpool,
            tc.tile_pool(name="xt", bufs=1) as xpool,
            tc.tile_pool(name="big", bufs=1) as bigpool,
            tc.tile_pool(name="vaug", bufs=NSQ) as vpool,
            tc.tile_pool(name="et", bufs=16) as epool,
            tc.tile_pool(name="y2", bufs=6) as ypool,
            tc.tile_pool(name="tmp", bufs=6) as tpool,
            tc.tile_pool(name="ot", bufs=3) as opool,
            tc.tile_pool(name="ps_a", bufs=2, space="PSUM") as ps_a,
            tc.tile_pool(name="ps_s", bufs=2, space="PSUM") as ps_s,
            tc.tile_pool(name="ps_av", bufs=2, space="PSUM") as ps_av,
        ):
            # ---- constants / weights / x in ----
            cos_sb = cpool.tile([P, S], f16, tag="cos")
            swap_sb = cpool.tile([P, S], f16, tag="swap")
            masks_sb = cpool.tile([P, P], f16, tag="masks")
            ident_sb = cpool.tile([HD, HD], f16, tag="ident")
            ident128_sb = cpool.tile([P, P], mm_dt, tag="ident128")
            zbias = cpool.tile([P, 1], f32, tag="zbias")
            nc.gpsimd.memset(zbias[:], 0.0)
            nc.gpsimd.dma_start(masks_sb[:], masks_d[:])
            nc.gpsimd.dma_start(ident_sb[:], ident_d[:])
            nc.gpsimd.dma_start(ident128_sb[:], ident128_d[:])



            # weights first (proj k=0 needs them), then x as column-slab
            # DMAs (one [all-k, 512-col] slab per half-chunk) so chunk j's
            # projection is fully fed by slab j.
            wq_sb = cpool.tile([P, NK * (QCOLS + KVCOLS)], mm_dt, tag="wqkv")
            wqv = wq_sb[:].rearrange("p (k c) -> p k c", k=NK)
            wqd = wqkvt_d[:].rearrange("(k p) c -> p k c", p=P)

            xt_big = xpool.tile([P, NK * S], mm_dt, tag="xt")
            xtv = xt_big[:].rearrange("p (k s) -> p k s", k=NK)
            xtd = xt_d[:].rearrange("(k p) s -> p k s", p=P)

            def xslab(q, ksplit=1):
                cs = slice(q * (S // 4), (q + 1) * (S // 4))
                kn = NK // ksplit
                for g in range(ksplit):
                    ks = slice(g * kn, (g + 1) * kn)
                    nc.sync.dma_start(xtv[:, ks, cs], xtd[:, ks, cs])

            cs0 = slice(0, S // 4)
            for g in range(4):
                ks = slice(4 * g, 4 * g + 4)
                nc.sync.dma_start(wqv[:, ks], wqd[:, ks])
                nc.sync.dma_start(xtv[:, ks, cs0], xtd[:, ks, cs0])
            # cos/swap after the slab-0 stream, on the SAME queue so they
            # don't steal DMA-device slots from the critical x feed; rows
            # are duplicated (0:64 == 64:128): DMA half, DVE-copy the rest
            nc.scalar.dma_start(cos_sb[0:HD, :], cos_d[0:HD, :])
            nc.scalar.dma_start(swap_sb[0:HD, :], swap_d[0:HD, :])
            nc.vector.tensor_copy(cos_sb[HD:P, :], cos_sb[0:HD, :])
            nc.vector.tensor_copy(swap_sb[HD:P, :], swap_sb[0:HD, :])
            xslab(1)
            wot_sb = []
            for k in range(2):
                t = cpool.tile([P, D], mm_dt, tag=f"wot{k}", name=f"wot{k}")
                nc.scalar.dma_start(t[:], wot_d[k * P : (k + 1) * P, :])
                wot_sb.append(t)
            xslab(2)
            xslab(3)
            xt_sb = [xtv[:, k, :] for k in range(NK)]

            # big persistent tensors
            qt_sb = [bigpool.tile([P, S], f16, tag=f"qt{m}", name=f"qt{m}") for m in range(2)]
            kt_sb = bigpool.tile([P, S], f16, tag="kt")
            vt_sb = bigpool.tile([HD, S], f16, tag="vt")
            yt_sb = bigpool.tile([P, 2 * S], mm_dt, tag="yt")  # [:, k*S + s]

            vaug_sb = [None] * NSQ
            y2_sb = {}

            def rope_math(dst, qr, chunk, rows):
                # qr holds the drained psum; rows = 64 (k) or 128 (2 heads)
                t2 = tpool.tile([rows, CH], f16, tag="rope_t2", name="rope_t2")
                for b in range(0, rows, HD):
                    nc.vector.tensor_mul(
                        t2[b : b + 32, :], qr[b + 32 : b + 64, :], swap_sb[b + 32 : b + 64, chunk]
                    )
                    nc.vector.tensor_mul(
                        t2[b + 32 : b + 64, :], qr[b : b + 32, :], swap_sb[b : b + 32, chunk]
                    )
                nc.vector.tensor_mul(dst[:, chunk], qr[0:rows, :], cos_sb[0:rows, chunk])
                nc.vector.tensor_add(dst[:, chunk], dst[:, chunk], t2[:])

            proj_ps = {}

            def proj_part(m, j, g):
                # granules 0-3: 4 contraction steps each; granule 3 also
                # drains psum (freeing the ps_a buf); granule 4: rope math
                chunk = slice(j * CH, (j + 1) * CH)
                if g == 4:
                    qr = proj_ps.pop((m, j))
                    with nc.named_scope("rope"):
                        if m < 2:
                            rope_math(qt_sb[m], qr, chunk, P)
                        else:
                            rope_math(kt_sb[0:HD, :], qr, chunk, HD)
                            nc.vector.tensor_copy(
                                kt_sb[HD:P, chunk], kt_sb[0:HD, chunk]
                            )
                    return
                if g == 0:
                    proj_ps[(m, j)] = ps_a.tile([P, CH], f32, tag="proj", name="ps_proj")
                ps = proj_ps[(m, j)]
                with nc.named_scope("proj"):
                    for k in range(4 * g, 4 * g + 4):
                        nc.tensor.matmul(
                            ps[:],
                            wqv[:, k, m * P : (m + 1) * P],
                            xt_sb[k][:, chunk],
                            start=(k == 0),
                            stop=(k == NK - 1),
                        )
                if g < 3:
                    return
                # drain: frees the psum buf with a single DVE read
                with nc.named_scope("rope"):
                    if m == 2:
                        nc.vector.tensor_copy(vt_sb[:, chunk], ps[HD:P, :])
                    qr = tpool.tile([P, CH], f16, tag="rope_qr", name="rope_qr")
                    nc.vector.tensor_copy(qr[0 : (HD if m == 2 else P), :],
                                          ps[0 : (HD if m == 2 else P), :])
                proj_ps[(m, j)] = qr

            def proj(m, j):
                for g in range(5):
                    proj_part(m, j, g)

            def proj_granules(m, j):
                return [lambda g=g: proj_part(m, j, g) for g in range(5)]

            def vtrans(j):
                with nc.named_scope("vtrans"):
                    for i in range(4 * j, 4 * j + 4):
                        pt = ps_av.tile([P, HD], f16, tag="av", name="ps_vt")
                        nc.tensor.transpose(
                            pt[:], vt_sb[:, i * P : (i + 1) * P], ident_sb[:]
                        )
                        va = vpool.tile([P, HD + 1], f16, tag="vaug", name=f"vaug{i}")
                        nc.vector.tensor_copy(va[:, 0:HD], pt[:])
                        nc.gpsimd.memset(va[:, HD : HD + 1], 1.0)
                        vaug_sb[i] = va

            def sdpa_head(j, h, fill=None, popn=1):
                # scores for head h over chunk j, AV interleaved pairwise
                hp, hh = h // 2, (h % 2) * HD
                nlive = 4 * j + 4
                offs = [max(0, (i - 4 * j)) * P for i in range(nlive)]
                ets = []
                # kt rows are duplicated (0:64 == 64:128) so both pair
                # elements read at the head's own row range — keeps the
                # stationary/moving partition bases equal for walrus.
                rg = slice(hh, hh + HD)
                py = ps_av.tile([P, 4 * (HD + 1)], f32, tag="av", name="ps_av")

                def av_chain(tq):
                    # one accumulation chain per q-tile (start..stop emitted
                    # contiguously: psum zero-regions allow only one pending
                    # group per bank)
                    t = 4 * j + tq
                    col = tq * (HD + 1)
                    with nc.named_scope("av"):
                        for i in range(t + 1):
                            nc.tensor.matmul(
                                py[:, col : col + HD + 1],
                                ets[i // 2][:, (i % 2) * CH + tq * P : (i % 2) * CH + (tq + 1) * P],
                                vaug_sb[i][:],
                                start=(i == 0),
                                stop=(i == t),
                            )

                def norm(tqs):
                    with nc.named_scope("norm"):
                        recip = tpool.tile([P, 4], f32, tag="recip", name="recip")
                        rv = py[:].rearrange("p (t c) -> p t c", t=4)
                        nc.vector.reciprocal(
                            recip[:, tqs[0] : tqs[-1] + 1], rv[:, tqs[0] : tqs[-1] + 1, HD]
                        )
                        if (hp, j) not in y2_sb:
                            y2_sb[(hp, j)] = ypool.tile(
                                [P, 4 * P], mm_dt, tag="y2", name=f"y2_{hp}_{j}"
                            )
                        y2 = y2_sb[(hp, j)]
                        for tq in tqs:
                            nc.vector.tensor_scalar_mul(
                                y2[:, tq * P + hh : tq * P + hh + HD],
                                py[:, tq * (HD + 1) : tq * (HD + 1) + HD],
                                recip[:, tq : tq + 1],
                            )

                for p in range(nlive // 2):
                    i = 2 * p
                    ps2 = ps_s.tile([P, 2 * CH], f32, tag="sc", name="ps_sc")
                    with nc.named_scope("scores"):
                        for u in range(2):
                            off = offs[i + u]
                            nc.tensor.matmul(
                                ps2[:, u * CH + off : (u + 1) * CH],
                                kt_sb[rg, (i + u) * P : (i + u + 1) * P],
                                qt_sb[hp][rg, j * CH + off : (j + 1) * CH],
                                start=True,
                                stop=True,
                            )
                    et2 = epool.tile([P, 2 * CH], f16, tag="et", name="et")
                    with nc.named_scope("exp"):
                        if offs[i] == 0 and offs[i + 1] == 0:
                            nc.scalar.activation(
                                et2[:],
                                ps2[:],
                                mybir.ActivationFunctionType.Exp,
                                bias=zbias[:],
                                scale=0.125,
                            )
                        else:
                            for u in range(2):
                                off = offs[i + u]
                                nc.scalar.activation(
                                    et2[:, u * CH + off : (u + 1) * CH],
                                    ps2[:, u * CH + off : (u + 1) * CH],
                                    mybir.ActivationFunctionType.Exp,
                                    bias=zbias[:],
                                    scale=0.125,
                                )
                    for u in range(2):
                        if i + u >= nlive - 4:  # diagonal tile
                            off = u * CH + offs[i + u]
                            with nc.named_scope("mask"):
                                # SBUF-only op: runs on the idle Pool engine
                                nc.gpsimd.tensor_mul(
                                    et2[:, off : off + P],
                                    et2[:, off : off + P],
                                    masks_sb[:],
                                )
                    ets.append(et2)
                    for _ in range(popn):
                        if fill:
                            fill.pop(0)()  # PE fill granules during exps
                    if j == 3 and h == 3 and p == nlive // 2 - 1:
                        # final head: q-tiles 0-1 of the last chunk are fully
                        # normalized; start their output projection while the
                        # last exp runs
                        for tq in (0, 1):
                            for dcJ in range(4):
                                wo_dch(3, tq, dcJ, last=True)
                    # emit each q-tile chain once its last pair is queued;
                    # earlier pairs' exps overlap the chain's early matmuls
                    for tq in range(4):
                        if (4 * j + tq) // 2 == p:
                            av_chain(tq)
                    if j == 3 and p == nlive // 2 - 2:
                        norm([0, 1])
                if j == 3:
                    norm([2, 3])
                else:
                    norm([0, 1, 2, 3])

            wo_ot = {}

            def wo_dch(j, tq, dcJ, last=False):
                # one [128, 512] granule of the output projection for
                # seq-tile 4j+tq; granule 0 also transposes y2 -> yT
                t = 4 * j + tq
                if dcJ == 0:
                    with nc.named_scope("ytrans"):
                        for hp in range(2):
                            y2 = y2_sb[(hp, j)]
                            dst = yt_sb[:, hp * S + t * P : hp * S + (t + 1) * P]
                            src = y2[:, tq * P : (tq + 1) * P]
                            pt = ps_av.tile([P, P], mm_dt, tag="av", name="ps_yt")
                            nc.tensor.transpose(pt[:], src, ident128_sb[:])
                            nc.vector.tensor_copy(dst, pt[:])
                    wo_ot[t] = opool.tile([P, D], mm_dt, tag="ot", name="ot")
                ot = wo_ot[t]
                srow = slice(t * P, (t + 1) * P)
                dch = slice(dcJ * CH, (dcJ + 1) * CH)
                pw = ps_a.tile([P, CH], f32, tag="proj", name="ps_wo")
                with nc.named_scope("wo"):
                    for k in range(2):
                        nc.tensor.matmul(
                            pw[:],
                            yt_sb[:, k * S + t * P : k * S + (t + 1) * P],
                            wot_sb[k][:, dch],
                            start=(k == 0),
                            stop=(k == 1),
                        )
                with nc.named_scope("outev"):
                    # gpsimd cannot touch psum on hw: evict on DVE, and on
                    # ACT for half the tail tiles (ACT is idle by then)
                    if last and (4 * tq + dcJ) % 2:
                        nc.scalar.activation(
                            ot[:, dch], pw[:],
                            mybir.ActivationFunctionType.Copy,
                            bias=0.0, scale=1.0,
                        )
                    else:
                        nc.vector.tensor_copy(ot[:, dch], pw[:])
                if last:  # overlap the final tiles' DMA with evictions,
                    # spread across HWDGE queues to avoid one-seq serialization
                    with nc.named_scope("outdma"):
                        q = [nc.sync, nc.scalar][(4 * tq + dcJ) % 2]
                        q.dma_start(out_d[srow, dch], ot[:, dch])
                elif dcJ == 3:
                    with nc.named_scope("outdma"):
                        nc.sync.dma_start(out_d[srow, :], ot[:])

            def wo_granules(j, last=False):
                return [
                    lambda tq=tq, dcJ=dcJ: wo_dch(j, tq, dcJ, last)
                    for tq in range(4)
                    for dcJ in range(4)
                ]

            # ---- main schedule: proj(j) -> sdpa(j) with proj(j+1)/wo(j-1)
            # units interleaved into the ACT-bound sdpa phase ----
            # kv and q01 of chunk 0 interleaved per k-granule (both gated
            # by the same x slab DMAs; ps_a has 2 bufs), then q23
            for g in range(4):
                proj_part(2, 0, g)
                proj_part(0, 0, g)
            proj_part(2, 0, 4)
            proj_part(0, 0, 4)
            vtrans(0)
            for j in range(NCH):
                # proj(1, j) deferred into chunk j itself (only heads 2-3
                # need it); wo lags so the ACT-heavy late chunks have PE
                # fill granules matched to their exp-wait stalls.
                fill = [] if j == 3 else proj_granules(1, j)
                if j + 1 < NCH:
                    fill += proj_granules(2, j + 1)
                    fill += [lambda jj=j: vtrans(jj + 1)]
                    fill += proj_granules(0, j + 1)
                if j == 2:
                    fill += proj_granules(1, 3) + wo_granules(0)
                if j == 3:
                    fill += wo_granules(1) + wo_granules(2)
                npops = QH * (2 * j + 2)
                popn = max(1, -(-len(fill) // npops))
                for h in range(QH):
                    sdpa_head(j, h, fill, popn)
                for f in fill:
                    f()
                fill.clear()
            for tq in (2, 3):
                for dcJ in range(4):
                    wo_dch(NCH - 1, tq, dcJ, last=True)

    nc.finalize()
    return nc


def _host_inputs(x, freqs_cos, freqs_sin, wq, wk, wv, wo):
    """Build the 8 per-core input maps (all host-side preprocessing)."""
    x = np.asarray(x, np.float32)
    cos = np.asarray(freqs_cos, np.float32)  # [S, 32]
    sin = np.asarray(freqs_sin, np.float32)
    wq = np.asarray(wq, np.float32)
    wk = np.asarray(wk, np.float32)
    wv = np.asarray(wv, np.float32)
    wo = np.asarray(wo, np.float32)

    perm = np.concatenate([np.arange(0, HD, 2), np.arange(1, HD, 2)])  # de-interleave

    xt = np.ascontiguousarray(x[0].T).astype(BF16)

    # cos128[d, t] = cos[t, d % 32] replicated to 128 rows (two heads per
    # rope pass); swap128 rows (b+0:b+32) = +sin, (b+32:b+64) = -sin
    cos128 = np.empty((P, S), np.float16)
    swap128 = np.empty((P, S), np.float16)
    for dd in range(P):
        i = dd % 32
        cos128[dd] = cos[:, i]
        swap128[dd] = sin[:, i] if (dd % HD) < 32 else -sin[:, i]

    pp = np.arange(P)[:, None]
    ff = np.arange(P)[None, :]
    masks = (pp <= ff).astype(np.float16)

    ident = np.eye(HD, dtype=np.float32).astype(np.float16)
    ident128 = np.eye(P, dtype=np.float32).astype(BF16)

    in_maps = []
    for c in range(NCORES):
        wq_c = wq[c * QCOLS : (c + 1) * QCOLS].reshape(QH, HD, D)[:, perm, :].reshape(
            QCOLS, D
        )
        wk_c = wk[c * HD : (c + 1) * HD][perm, :]
        wv_c = wv[c * HD : (c + 1) * HD]
        wqkvt = np.ascontiguousarray(
            np.concatenate([wq_c, wk_c, wv_c], axis=0).T
        ).astype(BF16)
        wot = np.ascontiguousarray(wo[:, c * QCOLS : (c + 1) * QCOLS].T).astype(BF16)
        in_maps.append(
            {
                "xt": xt,
                "wqkvt": wqkvt,
                "wot": wot,
                "cos128": cos128,
                "swap128": swap128,
                "masks": masks,
                "ident": ident,
                "ident128": ident128,
            }
        )
    return in_maps


def kernel(x, freqs_cos, freqs_sin, wq, wk, wv, wo):
    from concourse.bass_utils import run_bass_kernel_spmd

    if "nc" not in _CACHE:
        _CACHE["nc"] = _build()
    nc = _CACHE["nc"]
    in_maps = _host_inputs(x, freqs_cos, freqs_sin, wq, wk, wv, wo)
    res = run_bass_kernel_spmd(nc, in_maps, core_ids=list(range(NCORES)))
    out = np.zeros((S, D), np.float64)
    for r in res.results:
        out += r["out"].astype(np.float64)
    return out.astype(np.float32).reshape(1, S, D)


# revision 82
# speedup vs baseline: 1.0149x; 1.0149x over previous
"""GQA causal attention (llama3-style RoPE) on 8 TRN2 NeuronCores.

Sharding: tensor-parallel over heads. Core c gets q-heads 4c..4c+3 and
kv-head c (GQA groups intact), plus the matching row-block of wo.T.
Each core computes a full [S, D] partial of the output projection;
the host sums the 8 partials (the "all-reduce" of the row-sharded wo).

Per-core pipeline (bf16/f16 everywhere; layouts picked so the only
transpose is a cheap PE transpose of y):
  qkvT[col, s]   = wqkvT.T @ xT             (K-tiled bf16 matmuls)
  RoPE on qT/kT  (two heads per pass via 128-row replicated cos/sin)
  sT[sk, sq]     = kT.T @ qT                (K=64 f16, causal-trimmed)
  eT             = exp(sT/8) * causal_mask  (ACT exp, DVE diag mask)
  y[sq, 65]      = eT.T @ v_aug             (flipped AV: out free size is
                                             only 65, half the PE cycles of
                                             the [65, sq] orientation, and
                                             col 64 = softmax denominator)
  y2             = y[:, 0:64] * recip(y[:, 64])   (per-partition scalar)
  yT             = PE-transpose(y2)
  out[sq, d]     = yT.T @ woT               (partial; host sums cores)
"""

import sys

for _p in ("/opt/trn_rl_repo", "/root/.axon_site/_ro/trn_rl_repo"):
    if _p not in sys.path:
        sys.path.insert(0, _p)

import numpy as np
import ml_dtypes

import concourse.bass as bass
import concourse.bacc as bacc
import concourse.mybir as mybir
import concourse.tile as tile

BF16 = ml_dtypes.bfloat16

S = 2048
D = 2048
HD = 64
NH = 32
NKV = 8
NCORES = 8
QH = NH // NCORES            # 4 local q heads
QCOLS = QH * HD              # 256
KVCOLS = 2 * HD              # 128 (k and v, one kv head)
P = 128                      # partitions
NK = D // P                  # 16 contraction tiles
NSQ = S // P                 # 16 seq tiles of 128
NCH = 4                      # seq chunks of 512
CH = 512

_CACHE = {}


def _build():
    mm_dt = mybir.dt.bfloat16
    f16 = mybir.dt.float16
    f32 = mybir.dt.float32

    nc = bacc.Bacc()
    xt_d = nc.dram_tensor("xt", [D, S], mm_dt, kind="ExternalInput")
    wqkvt_d = nc.dram_tensor("wqkvt", [QCOLS + KVCOLS, D], mm_dt, kind="ExternalInput")
    wot_d = nc.dram_tensor("wot", [QCOLS, D], mm_dt, kind="ExternalInput")
    cos_d = nc.dram_tensor("cos128", [P, S], f16, kind="ExternalInput")
    swap_d = nc.dram_tensor("swap128", [P, S], f16, kind="ExternalInput")
    masks_d = nc.dram_tensor("masks", [P, P], f16, kind="ExternalInput")
    ident_d = nc.dram_tensor("ident", [HD, HD], f16, kind="ExternalInput")
    ident128_d = nc.dram_tensor("ident128", [P, P], mm_dt, kind="ExternalInput")
    out_d = nc.dram_tensor("out", [S, D], mm_dt, kind="ExternalOutput")

    with tile.TileContext(nc) as tc:
        with (
            tc.tile_pool(name="const", bufs=1) as cpool,
            tc.tile_pool(name="xt", bufs=1) as xpool,
            tc.tile_pool(name="big", bufs=1) as bigpool,
            tc.tile_pool(name="vaug", bufs=NSQ) as vpool,
            tc.tile_pool(name="et", bufs=16) as epool,
            tc.tile_pool(name="y2", bufs=6) as ypool,
            tc.tile_pool(name="tmp", bufs=6) as tpool,
            tc.tile_pool(name="ot", bufs=4) as opool,
            tc.tile_pool(name="ps_a", bufs=2, space="PSUM") as ps_a,
            tc.tile_pool(name="ps_s", bufs=2, space="PSUM") as ps_s,
            tc.tile_pool(name="ps_av", bufs=2, space="PSUM") as ps_av,
        ):
            # ---- constants / weights / x in ----
            cos_sb = cpool.tile([P, S], f16, tag="cos")
            swap_sb = cpool.tile([P, S], f16, tag="swap")
            masks_sb = cpool.tile([P, P], f16, tag="masks")
            ident_sb = cpool.tile([HD, HD], f16, tag="ident")
            ident128_sb = cpool.tile([P, P], mm_dt, tag="ident128")
            zbias = cpool.tile([P, 1], f32, tag="zbias")
            nc.gpsimd.memset(zbias[:], 0.0)
            nc.gpsimd.dma_start(masks_sb[:], masks_d[:])
            nc.gpsimd.dma_start(ident_sb[:], ident_d[:])
            nc.gpsimd.dma_start(ident128_sb[:], ident128_d[:])



            # weights first (proj k=0 needs them), then x as column-slab
            # DMAs (one [all-k, 512-col] slab per half-chunk) so chunk j's
            # projection is fully fed by slab j.
            # wqkvt is Mtile-major in dram ([3][128p][16k][128c]) so each
            # Mtile's weights arrive in one 4KB-row DMA; kv + q01 lead the
            # x slab stream, q23 trails (not needed until mid-chunk-0)
            wq_sb = cpool.tile([P, 3 * NK * P], mm_dt, tag="wqkv")
            wqv = wq_sb[:].rearrange("p (m k c) -> p m k c", m=3, k=NK)
            wqd = wqkvt_d[:].rearrange("(m p) (k c) -> p m k c", p=P, k=NK)

            xt_big = xpool.tile([P, NK * S], mm_dt, tag="xt")
            xtv = xt_big[:].rearrange("p (k s) -> p k s", k=NK)
            xtd = xt_d[:].rearrange("(k p) s -> p k s", p=P)

            def xslab(q, ksplit=1):
                cs = slice(q * (S // 4), (q + 1) * (S // 4))
                kn = NK // ksplit
                for g in range(ksplit):
                    ks = slice(g * kn, (g + 1) * kn)
                    nc.sync.dma_start(xtv[:, ks, cs], xtd[:, ks, cs])

            cs0 = slice(0, S // 4)
            nc.sync.dma_start(wqv[:, 2], wqd[:, 2])
            nc.sync.dma_start(xtv[:, 0:4, cs0], xtd[:, 0:4, cs0])
            nc.sync.dma_start(wqv[:, 0], wqd[:, 0])
            for g in range(1, 4):
                ks = slice(4 * g, 4 * g + 4)
                nc.sync.dma_start(xtv[:, ks, cs0], xtd[:, ks, cs0])
            nc.sync.dma_start(wqv[:, 1], wqd[:, 1])
            # cos/swap after the slab-0 stream, on the SAME queue so they
            # don't steal DMA-device slots from the critical x feed; rows
            # are duplicated (0:64 == 64:128): DMA half, DVE-copy the rest
            nc.scalar.dma_start(cos_sb[0:HD, :], cos_d[0:HD, :])
            nc.scalar.dma_start(swap_sb[0:HD, :], swap_d[0:HD, :])
            nc.vector.tensor_copy(cos_sb[HD:P, :], cos_sb[0:HD, :])
            nc.vector.tensor_copy(swap_sb[HD:P, :], swap_sb[0:HD, :])
            xslab(1)
            wot_sb = []
            for k in range(2):
                t = cpool.tile([P, D], mm_dt, tag=f"wot{k}", name=f"wot{k}")
                nc.scalar.dma_start(t[:], wot_d[k * P : (k + 1) * P, :])
                wot_sb.append(t)
            xslab(2)
            xslab(3)
            xt_sb = [xtv[:, k, :] for k in range(NK)]

            # big persistent tensors
            qt_sb = [bigpool.tile([P, S], f16, tag=f"qt{m}", name=f"qt{m}") for m in range(2)]
            kt_sb = bigpool.tile([P, S], f16, tag="kt")
            vt_sb = bigpool.tile([HD, S], f16, tag="vt")
            yt_sb = bigpool.tile([P, 2 * S], mm_dt, tag="yt")  # [:, k*S + s]

            vaug_sb = [None] * NSQ
            y2_sb = {}

            def rope_math(dst, qr, chunk, rows):
                # qr holds the drained psum; rows = 64 (k) or 128 (2 heads)
                t2 = tpool.tile([rows, CH], f16, tag="rope_t2", name="rope_t2")
                for b in range(0, rows, HD):
                    nc.vector.tensor_mul(
                        t2[b : b + 32, :], qr[b + 32 : b + 64, :], swap_sb[b + 32 : b + 64, chunk]
                    )
                    nc.vector.tensor_mul(
                        t2[b + 32 : b + 64, :], qr[b : b + 32, :], swap_sb[b : b + 32, chunk]
                    )
                nc.vector.tensor_mul(dst[:, chunk], qr[0:rows, :], cos_sb[0:rows, chunk])
                nc.vector.tensor_add(dst[:, chunk], dst[:, chunk], t2[:])

            proj_ps = {}

            def proj_part(m, j, g):
                # granules 0-3: 4 contraction steps each; granule 3 also
                # drains psum (freeing the ps_a buf); granule 4: rope math
                chunk = slice(j * CH, (j + 1) * CH)
                if g == 4:
                    qr = proj_ps.pop((m, j))
                    with nc.named_scope("rope"):
                        if m < 2:
                            rope_math(qt_sb[m], qr, chunk, P)
                        else:
                            rope_math(kt_sb[0:HD, :], qr, chunk, HD)
                            nc.vector.tensor_copy(
                                kt_sb[HD:P, chunk], kt_sb[0:HD, chunk]
                            )
                    return
                if g == 0:
                    proj_ps[(m, j)] = ps_a.tile([P, CH], f32, tag="proj", name="ps_proj")
                ps = proj_ps[(m, j)]
                with nc.named_scope("proj"):
                    for k in range(4 * g, 4 * g + 4):
                        nc.tensor.matmul(
                            ps[:],
                            wqv[:, m, k, :],
                            xt_sb[k][:, chunk],
                            start=(k == 0),
                            stop=(k == NK - 1),
                        )
                if g < 3:
                    return
                # drain: frees the psum buf with a single DVE read
                with nc.named_scope("rope"):
                    if m == 2:
                        nc.vector.tensor_copy(vt_sb[:, chunk], ps[HD:P, :])
                    qr = tpool.tile([P, CH], f16, tag="rope_qr", name="rope_qr")
                    nc.vector.tensor_copy(qr[0 : (HD if m == 2 else P), :],
                                          ps[0 : (HD if m == 2 else P), :])
                proj_ps[(m, j)] = qr

            def proj(m, j):
                for g in range(5):
                    proj_part(m, j, g)

            def proj_granules(m, j):
                return [lambda g=g: proj_part(m, j, g) for g in range(5)]

            def vtrans(j):
                with nc.named_scope("vtrans"):
                    for i in range(4 * j, 4 * j + 4):
                        pt = ps_av.tile([P, HD], f16, tag="av", name="ps_vt")
                        nc.tensor.transpose(
                            pt[:], vt_sb[:, i * P : (i + 1) * P], ident_sb[:]
                        )
                        va = vpool.tile([P, HD + 1], f16, tag="vaug", name=f"vaug{i}")
                        nc.vector.tensor_copy(va[:, 0:HD], pt[:])
                        nc.gpsimd.memset(va[:, HD : HD + 1], 1.0)
                        vaug_sb[i] = va

            def sdpa_head(j, h, fill=None, popn=1):
                # scores for head h over chunk j, AV interleaved pairwise
                hp, hh = h // 2, (h % 2) * HD
                nlive = 4 * j + 4
                offs = [max(0, (i - 4 * j)) * P for i in range(nlive)]
                ets = []
                # kt rows are duplicated (0:64 == 64:128) so both pair
                # elements read at the head's own row range — keeps the
                # stationary/moving partition bases equal for walrus.
                rg = slice(hh, hh + HD)
                py = ps_av.tile([P, 4 * (HD + 1)], f32, tag="av", name="ps_av")

                def av_chain(tq):
                    # one accumulation chain per q-tile (start..stop emitted
                    # contiguously: psum zero-regions allow only one pending
                    # group per bank)
                    t = 4 * j + tq
                    col = tq * (HD + 1)
                    with nc.named_scope("av"):
                        for i in range(t + 1):
                            nc.tensor.matmul(
                                py[:, col : col + HD + 1],
                                ets[i // 2][:, (i % 2) * CH + tq * P : (i % 2) * CH + (tq + 1) * P],
                                vaug_sb[i][:],
                                start=(i == 0),
                                stop=(i == t),
                            )

                def norm(tqs):
                    with nc.named_scope("norm"):
                        recip = tpool.tile([P, 4], f32, tag="recip", name="recip")
                        rv = py[:].rearrange("p (t c) -> p t c", t=4)
                        nc.vector.reciprocal(
                            recip[:, tqs[0] : tqs[-1] + 1], rv[:, tqs[0] : tqs[-1] + 1, HD]
                        )
                        if (hp, j) not in y2_sb:
                            y2_sb[(hp, j)] = ypool.tile(
                                [P, 4 * P], mm_dt, tag="y2", name=f"y2_{hp}_{j}"
                            )
                        y2 = y2_sb[(hp, j)]
                        for tq in tqs:
                            nc.vector.tensor_scalar_mul(
                                y2[:, tq * P + hh : tq * P + hh + HD],
                                py[:, tq * (HD + 1) : tq * (HD + 1) + HD],
                                recip[:, tq : tq + 1],
                            )

                for p in range(nlive // 2):
                    i = 2 * p
                    ps2 = ps_s.tile([P, 2 * CH], f32, tag="sc", name="ps_sc")
                    with nc.named_scope("scores"):
                        for u in range(2):
                            off = offs[i + u]
                            nc.tensor.matmul(
                                ps2[:, u * CH + off : (u + 1) * CH],
                                kt_sb[rg, (i + u) * P : (i + u + 1) * P],
                                qt_sb[hp][rg, j * CH + off : (j + 1) * CH],
                                start=True,
                                stop=True,
                            )
                    et2 = epool.tile([P, 2 * CH], f16, tag="et", name="et")
                    with nc.named_scope("exp"):
                        if offs[i] == 0 and offs[i + 1] == 0:
                            nc.scalar.activation(
                                et2[:],
                                ps2[:],
                                mybir.ActivationFunctionType.Exp,
                                bias=zbias[:],
                                scale=0.125,
                            )
                        else:
                            for u in range(2):
                                off = offs[i + u]
                                nc.scalar.activation(
                                    et2[:, u * CH + off : (u + 1) * CH],
                                    ps2[:, u * CH + off : (u + 1) * CH],
                                    mybir.ActivationFunctionType.Exp,
                                    bias=zbias[:],
                                    scale=0.125,
                                )
                    for u in range(2):
                        if i + u >= nlive - 4:  # diagonal tile
                            off = u * CH + offs[i + u]
                            with nc.named_scope("mask"):
                                # SBUF-only op: runs on the idle Pool engine
                                nc.gpsimd.tensor_mul(
                                    et2[:, off : off + P],
                                    et2[:, off : off + P],
                                    masks_sb[:],
                                )
                    ets.append(et2)
                    for _ in range(popn):
                        if fill:
                            fill.pop(0)()  # PE fill granules during exps
                    if j == 3 and h == 3 and p == nlive // 2 - 1:
                        # final head: q-tiles 0-1 of the last chunk are fully
                        # normalized; start their output projection while the
                        # last exp runs
                        for tq in (0, 1):
                            for dcJ in range(4):
                                wo_dch(3, tq, dcJ, last=True)
                    # emit each q-tile chain once its last pair is queued;
                    # earlier pairs' exps overlap the chain's early matmuls
                    for tq in range(4):
                        if (4 * j + tq) // 2 == p:
                            av_chain(tq)
                    if j == 3 and p == nlive // 2 - 2:
                        norm([0, 1])
                if j == 3:
                    norm([2, 3])
                else:
                    norm([0, 1, 2, 3])

            wo_ot = {}

            def wo_dch(j, tq, dcJ, last=False):
                # one [128, 512] granule of the output projection for
                # seq-tile 4j+tq; granule 0 also transposes y2 -> yT
                t = 4 * j + tq
                if dcJ == 0:
                    with nc.named_scope("ytrans"):
                        for hp in range(2):
                            y2 = y2_sb[(hp, j)]
                            dst = yt_sb[:, hp * S + t * P : hp * S + (t + 1) * P]
                            src = y2[:, tq * P : (tq + 1) * P]
                            pt = ps_av.tile([P, P], mm_dt, tag="av", name="ps_yt")
                            nc.tensor.transpose(pt[:], src, ident128_sb[:])
                            nc.vector.tensor_copy(dst, pt[:])
                    wo_ot[t] = opool.tile([P, D], mm_dt, tag="ot", name="ot")
                ot = wo_ot[t]
                srow = slice(t * P, (t + 1) * P)
                dch = slice(dcJ * CH, (dcJ + 1) * CH)
                pw = ps_a.tile([P, CH], f32, tag="proj", name="ps_wo")
                with nc.named_scope("wo"):
                    for k in range(2):
                        nc.tensor.matmul(
                            pw[:],
                            yt_sb[:, k * S + t * P : k * S + (t + 1) * P],
                            wot_sb[k][:, dch],
                            start=(k == 0),
                            stop=(k == 1),
                        )
                with nc.named_scope("outev"):
                    # gpsimd cannot touch psum on hw: evict on DVE, and on
                    # ACT for half the tail tiles (ACT is idle by then)
                    if last and (4 * tq + dcJ) % 2:
                        nc.scalar.activation(
                            ot[:, dch], pw[:],
                            mybir.ActivationFunctionType.Copy,
                            bias=0.0, scale=1.0,
                        )
                    else:
                        nc.vector.tensor_copy(ot[:, dch], pw[:])
                if last:  # overlap the final tiles' DMA with evictions,
                    # spread across HWDGE queues to avoid one-seq serialization
                    with nc.named_scope("outdma"):
                        q = [nc.sync, nc.scalar][(4 * tq + dcJ) % 2]
                        q.dma_start(out_d[srow, dch], ot[:, dch])
                elif dcJ == 3:
                    with nc.named_scope("outdma"):
                        nc.sync.dma_start(out_d[srow, :], ot[:])

            def wo_granules(j, last=False):
                return [
                    lambda tq=tq, dcJ=dcJ: wo_dch(j, tq, dcJ, last)
                    for tq in range(4)
                    for dcJ in range(4)
                ]

            # ---- main schedule: proj(j) -> sdpa(j) with proj(j+1)/wo(j-1)
            # units interleaved into the ACT-bound sdpa phase ----
            # kv and q01 of chunk 0 interleaved per k-granule (both gated
            # by the same x slab DMAs; ps_a has 2 bufs), then q23
            for g in range(4):
                proj_part(2, 0, g)
                proj_part(0, 0, g)
            proj_part(2, 0, 4)
            proj_part(0, 0, 4)
            vtrans(0)
            for j in range(NCH):
                # proj(1, j) deferred into chunk j itself (only heads 2-3
                # need it); wo lags so the ACT-heavy late chunks have PE
                # fill granules matched to their exp-wait stalls.
                fill = [] if j == 3 else proj_granules(1, j)
                if j + 1 < NCH:
                    fill += proj_granules(2, j + 1)
                    fill += [lambda jj=j: vtrans(jj + 1)]
                    fill += proj_granules(0, j + 1)
                if j == 2:
                    fill += proj_granules(1, 3) + wo_granules(0)
                if j == 3:
                    fill += wo_granules(1) + wo_granules(2)
                npops = QH * (2 * j + 2)
                popn = max(1, -(-len(fill) // npops))
                for h in range(QH):
                    sdpa_head(j, h, fill, popn)
                for f in fill:
                    f()
                fill.clear()
            for tq in (2, 3):
                for dcJ in range(4):
                    wo_dch(NCH - 1, tq, dcJ, last=True)

    nc.finalize()
    return nc


def _host_inputs(x, freqs_cos, freqs_sin, wq, wk, wv, wo):
    """Build the 8 per-core input maps (all host-side preprocessing)."""
    x = np.asarray(x, np.float32)
    cos = np.asarray(freqs_cos, np.float32)  # [S, 32]
    sin = np.asarray(freqs_sin, np.float32)
    wq = np.asarray(wq, np.float32)
    wk = np.asarray(wk, np.float32)
    wv = np.asarray(wv, np.float32)
    wo = np.asarray(wo, np.float32)

    perm = np.concatenate([np.arange(0, HD, 2), np.arange(1, HD, 2)])  # de-interleave

    xt = np.ascontiguousarray(x[0].T).astype(BF16)

    # cos128[d, t] = cos[t, d % 32] replicated to 128 rows (two heads per
    # rope pass); swap128 rows (b+0:b+32) = +sin, (b+32:b+64) = -sin
    cos128 = np.empty((P, S), np.float16)
    swap128 = np.empty((P, S), np.float16)
    for dd in range(P):
        i = dd % 32
        cos128[dd] = cos[:, i]
        swap128[dd] = sin[:, i] if (dd % HD) < 32 else -sin[:, i]

    pp = np.arange(P)[:, None]
    ff = np.arange(P)[None, :]
    masks = (pp <= ff).astype(np.float16)

    ident = np.eye(HD, dtype=np.float32).astype(np.float16)
    ident128 = np.eye(P, dtype=np.float32).astype(BF16)

    in_maps = []
    for c in range(NCORES):
        wq_c = wq[c * QCOLS : (c + 1) * QCOLS].reshape(QH, HD, D)[:, perm, :].reshape(
            QCOLS, D
        )
        wk_c = wk[c * HD : (c + 1) * HD][perm, :]
        wv_c = wv[c * HD : (c + 1) * HD]
        # Mtile-major: [3 m][128 p][16 k][128 c] flattened to [384, 2048]
        wqkvt = np.ascontiguousarray(
            np.concatenate([wq_c, wk_c, wv_c], axis=0)
            .T.reshape(NK, P, 3, P)
            .transpose(2, 1, 0, 3)
            .reshape(3 * P, NK * P)
        ).astype(BF16)
        wot = np.ascontiguousarray(wo[:, c * QCOLS : (c + 1) * QCOLS].T).astype(BF16)
        in_maps.append(
            {
                "xt": xt,
                "wqkvt": wqkvt,
                "wot": wot,
                "cos128": cos128,
                "swap128": swap128,
                "masks": masks,
                "ident": ident,
                "ident128": ident128,
            }
        )
    return in_maps


def kernel(x, freqs_cos, freqs_sin, wq, wk, wv, wo):
    from concourse.bass_utils import run_bass_kernel_spmd

    if "nc" not in _CACHE:
        _CACHE["nc"] = _build()
    nc = _CACHE["nc"]
    in_maps = _host_inputs(x, freqs_cos, freqs_sin, wq, wk, wv, wo)
    res = run_bass_kernel_spmd(nc, in_maps, core_ids=list(range(NCORES)))
    out = np.zeros((S, D), np.float64)
    for r in res.results:
        out += r["out"].astype(np.float64)
    return out.astype(np.float32).reshape(1, S, D)


# revision 85
# speedup vs baseline: 1.0162x; 1.0012x over previous
"""GQA causal attention (llama3-style RoPE) on 8 TRN2 NeuronCores.

Sharding: tensor-parallel over heads. Core c gets q-heads 4c..4c+3 and
kv-head c (GQA groups intact), plus the matching row-block of wo.T.
Each core computes a full [S, D] partial of the output projection;
the host sums the 8 partials (the "all-reduce" of the row-sharded wo).

Per-core pipeline (bf16/f16 everywhere; layouts picked so the only
transpose is a cheap PE transpose of y):
  qkvT[col, s]   = wqkvT.T @ xT             (K-tiled bf16 matmuls)
  RoPE on qT/kT  (two heads per pass via 128-row replicated cos/sin)
  sT[sk, sq]     = kT.T @ qT                (K=64 f16, causal-trimmed)
  eT             = exp(sT/8) * causal_mask  (ACT exp, DVE diag mask)
  y[sq, 65]      = eT.T @ v_aug             (flipped AV: out free size is
                                             only 65, half the PE cycles of
                                             the [65, sq] orientation, and
                                             col 64 = softmax denominator)
  y2             = y[:, 0:64] * recip(y[:, 64])   (per-partition scalar)
  yT             = PE-transpose(y2)
  out[sq, d]     = yT.T @ woT               (partial; host sums cores)
"""

import sys

for _p in ("/opt/trn_rl_repo", "/root/.axon_site/_ro/trn_rl_repo"):
    if _p not in sys.path:
        sys.path.insert(0, _p)

import numpy as np
import ml_dtypes

import concourse.bass as bass
import concourse.bacc as bacc
import concourse.mybir as mybir
import concourse.tile as tile

BF16 = ml_dtypes.bfloat16

S = 2048
D = 2048
HD = 64
NH = 32
NKV = 8
NCORES = 8
QH = NH // NCORES            # 4 local q heads
QCOLS = QH * HD              # 256
KVCOLS = 2 * HD              # 128 (k and v, one kv head)
P = 128                      # partitions
NK = D // P                  # 16 contraction tiles
NSQ = S // P                 # 16 seq tiles of 128
NCH = 4                      # seq chunks of 512
CH = 512

_CACHE = {}


def _build():
    mm_dt = mybir.dt.bfloat16
    f16 = mybir.dt.float16
    f32 = mybir.dt.float32

    nc = bacc.Bacc()
    xt_d = nc.dram_tensor("xt", [D, S], mm_dt, kind="ExternalInput")
    wqkvt_d = nc.dram_tensor("wqkvt", [QCOLS + KVCOLS, D], mm_dt, kind="ExternalInput")
    wot_d = nc.dram_tensor("wot", [QCOLS, D], mm_dt, kind="ExternalInput")
    cos_d = nc.dram_tensor("cos128", [P, S], f16, kind="ExternalInput")
    swap_d = nc.dram_tensor("swap128", [P, S], f16, kind="ExternalInput")
    masks_d = nc.dram_tensor("masks", [P, P], f16, kind="ExternalInput")
    ident_d = nc.dram_tensor("ident", [HD, HD], f16, kind="ExternalInput")
    ident128_d = nc.dram_tensor("ident128", [P, P], mm_dt, kind="ExternalInput")
    out_d = nc.dram_tensor("out", [S, D], mm_dt, kind="ExternalOutput")

    with tile.TileContext(nc) as tc:
        with (
            tc.tile_pool(name="const", bufs=1) as cpool,
            tc.tile_pool(name="xt", bufs=1) as xpool,
            tc.tile_pool(name="big", bufs=1) as bigpool,
            tc.tile_pool(name="vaug", bufs=NSQ) as vpool,
            tc.tile_pool(name="et", bufs=16) as epool,
            tc.tile_pool(name="y2", bufs=6) as ypool,
            tc.tile_pool(name="tmp", bufs=6) as tpool,
            tc.tile_pool(name="ot", bufs=4) as opool,
            tc.tile_pool(name="ps_a", bufs=2, space="PSUM") as ps_a,
            tc.tile_pool(name="ps_s", bufs=2, space="PSUM") as ps_s,
            tc.tile_pool(name="ps_av", bufs=2, space="PSUM") as ps_av,
        ):
            # ---- constants / weights / x in ----
            cos_sb = cpool.tile([P, S], f16, tag="cos")
            swap_sb = cpool.tile([P, S], f16, tag="swap")
            masks_sb = cpool.tile([P, P], f16, tag="masks")
            ident_sb = cpool.tile([HD, HD], f16, tag="ident")
            ident128_sb = cpool.tile([P, P], mm_dt, tag="ident128")
            zbias = cpool.tile([P, 1], f32, tag="zbias")
            nc.gpsimd.memset(zbias[:], 0.0)
            nc.gpsimd.dma_start(masks_sb[:], masks_d[:])
            nc.gpsimd.dma_start(ident_sb[:], ident_d[:])
            nc.gpsimd.dma_start(ident128_sb[:], ident128_d[:])



            # weights first (proj k=0 needs them), then x as column-slab
            # DMAs (one [all-k, 512-col] slab per half-chunk) so chunk j's
            # projection is fully fed by slab j.
            # wqkvt is Mtile-major in dram ([3][128p][16k][128c]) so each
            # Mtile's weights arrive in one 4KB-row DMA; kv + q01 lead the
            # x slab stream, q23 trails (not needed until mid-chunk-0)
            wq_sb = cpool.tile([P, 3 * NK * P], mm_dt, tag="wqkv")
            wqv = wq_sb[:].rearrange("p (m k c) -> p m k c", m=3, k=NK)
            wqd = wqkvt_d[:].rearrange("(m p) (k c) -> p m k c", p=P, k=NK)

            xt_big = xpool.tile([P, NK * S], mm_dt, tag="xt")
            xtv = xt_big[:].rearrange("p (k s) -> p k s", k=NK)
            xtd = xt_d[:].rearrange("(k p) s -> p k s", p=P)

            def xslab(q, ksplit=1):
                cs = slice(q * (S // 4), (q + 1) * (S // 4))
                kn = NK // ksplit
                for g in range(ksplit):
                    ks = slice(g * kn, (g + 1) * kn)
                    nc.sync.dma_start(xtv[:, ks, cs], xtd[:, ks, cs])

            cs0 = slice(0, S // 4)
            nc.sync.dma_start(wqv[:, 2], wqd[:, 2])
            nc.sync.dma_start(xtv[:, 0:4, cs0], xtd[:, 0:4, cs0])
            nc.sync.dma_start(wqv[:, 0], wqd[:, 0])
            for g in range(1, 4):
                ks = slice(4 * g, 4 * g + 4)
                nc.sync.dma_start(xtv[:, ks, cs0], xtd[:, ks, cs0])
            nc.sync.dma_start(wqv[:, 1], wqd[:, 1])
            # cos/swap after the slab-0 stream, on the SAME queue so they
            # don't steal DMA-device slots from the critical x feed; rows
            # are duplicated (0:64 == 64:128): DMA half, DVE-copy the rest
            nc.scalar.dma_start(cos_sb[0:HD, :], cos_d[0:HD, :])
            nc.scalar.dma_start(swap_sb[0:HD, :], swap_d[0:HD, :])
            nc.vector.tensor_copy(cos_sb[HD:P, :], cos_sb[0:HD, :])
            nc.vector.tensor_copy(swap_sb[HD:P, :], swap_sb[0:HD, :])
            xslab(1)
            wot_sb = []
            for k in range(2):
                t = cpool.tile([P, D], mm_dt, tag=f"wot{k}", name=f"wot{k}")
                nc.scalar.dma_start(t[:], wot_d[k * P : (k + 1) * P, :])
                wot_sb.append(t)
            xslab(2)
            xslab(3)
            xt_sb = [xtv[:, k, :] for k in range(NK)]

            # big persistent tensors
            qt_sb = [bigpool.tile([P, S], f16, tag=f"qt{m}", name=f"qt{m}") for m in range(2)]
            kt_sb = bigpool.tile([P, S], f16, tag="kt")
            vt_sb = bigpool.tile([HD, S], f16, tag="vt")
            yt_sb = bigpool.tile([P, 2 * S], mm_dt, tag="yt")  # [:, k*S + s]

            vaug_sb = [None] * NSQ
            y2_sb = {}

            def rope_math(dst, qr, chunk, rows):
                # qr holds the drained psum; rows = 64 (k) or 128 (2 heads)
                t2 = tpool.tile([rows, CH], f16, tag="rope_t2", name="rope_t2")
                for b in range(0, rows, HD):
                    nc.vector.tensor_mul(
                        t2[b : b + 32, :], qr[b + 32 : b + 64, :], swap_sb[b + 32 : b + 64, chunk]
                    )
                    nc.vector.tensor_mul(
                        t2[b + 32 : b + 64, :], qr[b : b + 32, :], swap_sb[b : b + 32, chunk]
                    )
                nc.vector.tensor_mul(dst[:, chunk], qr[0:rows, :], cos_sb[0:rows, chunk])
                nc.vector.tensor_add(dst[:, chunk], dst[:, chunk], t2[:])

            proj_ps = {}

            def proj_part(m, j, g):
                # granules 0-3: 4 contraction steps each; granule 3 also
                # drains psum (freeing the ps_a buf); granule 4: rope math
                chunk = slice(j * CH, (j + 1) * CH)
                if g == 8:
                    qr = proj_ps.pop((m, j))
                    with nc.named_scope("rope"):
                        if m < 2:
                            rope_math(qt_sb[m], qr, chunk, P)
                        else:
                            rope_math(kt_sb[0:HD, :], qr, chunk, HD)
                            nc.vector.tensor_copy(
                                kt_sb[HD:P, chunk], kt_sb[0:HD, chunk]
                            )
                    return
                if g == 0:
                    proj_ps[(m, j)] = ps_a.tile([P, CH], f32, tag="proj", name="ps_proj")
                ps = proj_ps[(m, j)]
                with nc.named_scope("proj"):
                    for k in range(2 * g, 2 * g + 2):
                        nc.tensor.matmul(
                            ps[:],
                            wqv[:, m, k, :],
                            xt_sb[k][:, chunk],
                            start=(k == 0),
                            stop=(k == NK - 1),
                        )
                if g < 7:
                    return
                # drain: frees the psum buf with a single DVE read
                with nc.named_scope("rope"):
                    if m == 2:
                        nc.vector.tensor_copy(vt_sb[:, chunk], ps[HD:P, :])
                    qr = tpool.tile([P, CH], f16, tag="rope_qr", name="rope_qr")
                    nc.vector.tensor_copy(qr[0 : (HD if m == 2 else P), :],
                                          ps[0 : (HD if m == 2 else P), :])
                proj_ps[(m, j)] = qr

            def proj(m, j):
                for g in range(9):
                    proj_part(m, j, g)

            def proj_granules(m, j):
                return [lambda g=g: proj_part(m, j, g) for g in range(9)]

            def vtrans(j):
                with nc.named_scope("vtrans"):
                    for i in range(4 * j, 4 * j + 4):
                        pt = ps_av.tile([P, HD], f16, tag="av", name="ps_vt")
                        nc.tensor.transpose(
                            pt[:], vt_sb[:, i * P : (i + 1) * P], ident_sb[:]
                        )
                        va = vpool.tile([P, HD + 1], f16, tag="vaug", name=f"vaug{i}")
                        nc.vector.tensor_copy(va[:, 0:HD], pt[:])
                        nc.gpsimd.memset(va[:, HD : HD + 1], 1.0)
                        vaug_sb[i] = va

            def sdpa_head(j, h, fill=None, popn=1):
                # scores for head h over chunk j, AV interleaved pairwise
                hp, hh = h // 2, (h % 2) * HD
                nlive = 4 * j + 4
                offs = [max(0, (i - 4 * j)) * P for i in range(nlive)]
                ets = []
                # kt rows are duplicated (0:64 == 64:128) so both pair
                # elements read at the head's own row range — keeps the
                # stationary/moving partition bases equal for walrus.
                rg = slice(hh, hh + HD)
                py = ps_av.tile([P, 4 * (HD + 1)], f32, tag="av", name="ps_av")

                def av_chain(tq):
                    # one accumulation chain per q-tile (start..stop emitted
                    # contiguously: psum zero-regions allow only one pending
                    # group per bank)
                    t = 4 * j + tq
                    col = tq * (HD + 1)
                    with nc.named_scope("av"):
                        for i in range(t + 1):
                            nc.tensor.matmul(
                                py[:, col : col + HD + 1],
                                ets[i // 2][:, (i % 2) * CH + tq * P : (i % 2) * CH + (tq + 1) * P],
                                vaug_sb[i][:],
                                start=(i == 0),
                                stop=(i == t),
                            )

                def norm(tqs):
                    with nc.named_scope("norm"):
                        recip = tpool.tile([P, 4], f32, tag="recip", name="recip")
                        rv = py[:].rearrange("p (t c) -> p t c", t=4)
                        nc.vector.reciprocal(
                            recip[:, tqs[0] : tqs[-1] + 1], rv[:, tqs[0] : tqs[-1] + 1, HD]
                        )
                        if (hp, j) not in y2_sb:
                            y2_sb[(hp, j)] = ypool.tile(
                                [P, 4 * P], mm_dt, tag="y2", name=f"y2_{hp}_{j}"
                            )
                        y2 = y2_sb[(hp, j)]
                        for tq in tqs:
                            nc.vector.tensor_scalar_mul(
                                y2[:, tq * P + hh : tq * P + hh + HD],
                                py[:, tq * (HD + 1) : tq * (HD + 1) + HD],
                                recip[:, tq : tq + 1],
                            )

                for p in range(nlive // 2):
                    i = 2 * p
                    ps2 = ps_s.tile([P, 2 * CH], f32, tag="sc", name="ps_sc")
                    with nc.named_scope("scores"):
                        for u in range(2):
                            off = offs[i + u]
                            nc.tensor.matmul(
                                ps2[:, u * CH + off : (u + 1) * CH],
                                kt_sb[rg, (i + u) * P : (i + u + 1) * P],
                                qt_sb[hp][rg, j * CH + off : (j + 1) * CH],
                                start=True,
                                stop=True,
                            )
                    et2 = epool.tile([P, 2 * CH], f16, tag="et", name="et")
                    with nc.named_scope("exp"):
                        if offs[i] == 0 and offs[i + 1] == 0:
                            nc.scalar.activation(
                                et2[:],
                                ps2[:],
                                mybir.ActivationFunctionType.Exp,
                                bias=zbias[:],
                                scale=0.125,
                            )
                        else:
                            for u in range(2):
                                off = offs[i + u]
                                nc.scalar.activation(
                                    et2[:, u * CH + off : (u + 1) * CH],
                                    ps2[:, u * CH + off : (u + 1) * CH],
                                    mybir.ActivationFunctionType.Exp,
                                    bias=zbias[:],
                                    scale=0.125,
                                )
                    for u in range(2):
                        if i + u >= nlive - 4:  # diagonal tile
                            off = u * CH + offs[i + u]
                            with nc.named_scope("mask"):
                                # SBUF-only op: runs on the idle Pool engine
                                nc.gpsimd.tensor_mul(
                                    et2[:, off : off + P],
                                    et2[:, off : off + P],
                                    masks_sb[:],
                                )
                    ets.append(et2)
                    for _ in range(popn):
                        if fill:
                            fill.pop(0)()  # PE fill granules during exps
                    if j == 3 and h == 3 and p == nlive // 2 - 1:
                        # final head: q-tiles 0-1 of the last chunk are fully
                        # normalized; start their output projection while the
                        # last exp runs
                        for tq in (0, 1):
                            for dcJ in range(4):
                                wo_dch(3, tq, dcJ, last=True)
                    # emit each q-tile chain once its last pair is queued;
                    # earlier pairs' exps overlap the chain's early matmuls
                    for tq in range(4):
                        if (4 * j + tq) // 2 == p:
                            av_chain(tq)
                    if j == 3 and p == nlive // 2 - 2:
                        norm([0, 1])
                if j == 3:
                    norm([2, 3])
                else:
                    norm([0, 1, 2, 3])

            wo_ot = {}

            def wo_dch(j, tq, dcJ, last=False):
                # one [128, 512] granule of the output projection for
                # seq-tile 4j+tq; granule 0 also transposes y2 -> yT
                t = 4 * j + tq
                if dcJ == 0:
                    with nc.named_scope("ytrans"):
                        for hp in range(2):
                            y2 = y2_sb[(hp, j)]
                            dst = yt_sb[:, hp * S + t * P : hp * S + (t + 1) * P]
                            src = y2[:, tq * P : (tq + 1) * P]
                            pt = ps_av.tile([P, P], mm_dt, tag="av", name="ps_yt")
                            nc.tensor.transpose(pt[:], src, ident128_sb[:])
                            nc.vector.tensor_copy(dst, pt[:])
                    wo_ot[t] = opool.tile([P, D], mm_dt, tag="ot", name="ot")
                ot = wo_ot[t]
                srow = slice(t * P, (t + 1) * P)
                dch = slice(dcJ * CH, (dcJ + 1) * CH)
                pw = ps_a.tile([P, CH], f32, tag="proj", name="ps_wo")
                with nc.named_scope("wo"):
                    for k in range(2):
                        nc.tensor.matmul(
                            pw[:],
                            yt_sb[:, k * S + t * P : k * S + (t + 1) * P],
                            wot_sb[k][:, dch],
                            start=(k == 0),
                            stop=(k == 1),
                        )
                with nc.named_scope("outev"):
                    # gpsimd cannot touch psum on hw: evict on DVE, and on
                    # ACT for half the tail tiles (ACT is idle by then)
                    if last and (4 * tq + dcJ) % 2:
                        nc.scalar.activation(
                            ot[:, dch], pw[:],
                            mybir.ActivationFunctionType.Copy,
                            bias=0.0, scale=1.0,
                        )
                    else:
                        nc.vector.tensor_copy(ot[:, dch], pw[:])
                if last:  # overlap the final tiles' DMA with evictions,
                    # spread across HWDGE queues to avoid one-seq serialization
                    with nc.named_scope("outdma"):
                        q = [nc.sync, nc.scalar][(4 * tq + dcJ) % 2]
                        q.dma_start(out_d[srow, dch], ot[:, dch])
                elif dcJ == 3:
                    with nc.named_scope("outdma"):
                        nc.sync.dma_start(out_d[srow, :], ot[:])

            def wo_granules(j, last=False):
                return [
                    lambda tq=tq, dcJ=dcJ: wo_dch(j, tq, dcJ, last)
                    for tq in range(4)
                    for dcJ in range(4)
                ]

            # ---- main schedule: proj(j) -> sdpa(j) with proj(j+1)/wo(j-1)
            # units interleaved into the ACT-bound sdpa phase ----
            # kv and q01 of chunk 0 interleaved per k-granule (both gated
            # by the same x slab DMAs; ps_a has 2 bufs), then q23
            for g in range(8):
                proj_part(2, 0, g)
                proj_part(0, 0, g)
            proj_part(2, 0, 8)
            proj_part(0, 0, 8)
            vtrans(0)
            for j in range(NCH):
                # proj(1, j) deferred into chunk j itself (only heads 2-3
                # need it); wo lags so the ACT-heavy late chunks have PE
                # fill granules matched to their exp-wait stalls.
                fill = [] if j == 3 else proj_granules(1, j)
                if j + 1 < NCH:
                    fill += proj_granules(2, j + 1)
                    fill += [lambda jj=j: vtrans(jj + 1)]
                    fill += proj_granules(0, j + 1)
                if j == 2:
                    fill += proj_granules(1, 3) + wo_granules(0)
                if j == 3:
                    fill += wo_granules(1) + wo_granules(2)
                npops = QH * (2 * j + 2)
                popn = max(1, -(-len(fill) // npops))
                for h in range(QH):
                    sdpa_head(j, h, fill, popn)
                for f in fill:
                    f()
                fill.clear()
            for tq in (2, 3):
                for dcJ in range(4):
                    wo_dch(NCH - 1, tq, dcJ, last=True)

    nc.finalize()
    return nc


def _host_inputs(x, freqs_cos, freqs_sin, wq, wk, wv, wo):
    """Build the 8 per-core input maps (all host-side preprocessing)."""
    x = np.asarray(x, np.float32)
    cos = np.asarray(freqs_cos, np.float32)  # [S, 32]
    sin = np.asarray(freqs_sin, np.float32)
    wq = np.asarray(wq, np.float32)
    wk = np.asarray(wk, np.float32)
    wv = np.asarray(wv, np.float32)
    wo = np.asarray(wo, np.float32)

    perm = np.concatenate([np.arange(0, HD, 2), np.arange(1, HD, 2)])  # de-interleave

    xt = np.ascontiguousarray(x[0].T).astype(BF16)

    # cos128[d, t] = cos[t, d % 32] replicated to 128 rows (two heads per
    # rope pass); swap128 rows (b+0:b+32) = +sin, (b+32:b+64) = -sin
    cos128 = np.empty((P, S), np.float16)
    swap128 = np.empty((P, S), np.float16)
    for dd in range(P):
        i = dd % 32
        cos128[dd] = cos[:, i]
        swap128[dd] = sin[:, i] if (dd % HD) < 32 else -sin[:, i]

    pp = np.arange(P)[:, None]
    ff = np.arange(P)[None, :]
    masks = (pp <= ff).astype(np.float16)

    ident = np.eye(HD, dtype=np.float32).astype(np.float16)
    ident128 = np.eye(P, dtype=np.float32).astype(BF16)

    in_maps = []
    for c in range(NCORES):
        wq_c = wq[c * QCOLS : (c + 1) * QCOLS].reshape(QH, HD, D)[:, perm, :].reshape(
            QCOLS, D
        )
        wk_c = wk[c * HD : (c + 1) * HD][perm, :]
        wv_c = wv[c * HD : (c + 1) * HD]
        # Mtile-major: [3 m][128 p][16 k][128 c] flattened to [384, 2048]
        wqkvt = np.ascontiguousarray(
            np.concatenate([wq_c, wk_c, wv_c], axis=0)
            .T.reshape(NK, P, 3, P)
            .transpose(2, 1, 0, 3)
            .reshape(3 * P, NK * P)
        ).astype(BF16)
        wot = np.ascontiguousarray(wo[:, c * QCOLS : (c + 1) * QCOLS].T).astype(BF16)
        in_maps.append(
            {
                "xt": xt,
                "wqkvt": wqkvt,
                "wot": wot,
                "cos128": cos128,
                "swap128": swap128,
                "masks": masks,
                "ident": ident,
                "ident128": ident128,
            }
        )
    return in_maps


def kernel(x, freqs_cos, freqs_sin, wq, wk, wv, wo):
    from concourse.bass_utils import run_bass_kernel_spmd

    if "nc" not in _CACHE:
        _CACHE["nc"] = _build()
    nc = _CACHE["nc"]
    in_maps = _host_inputs(x, freqs_cos, freqs_sin, wq, wk, wv, wo)
    res = run_bass_kernel_spmd(nc, in_maps, core_ids=list(range(NCORES)))
    out = np.zeros((S, D), np.float64)
    for r in res.results:
        out += r["out"].astype(np.float64)
    return out.astype(np.float32).reshape(1, S, D)
